# revision 22
# baseline (speedup 1.0000x reference)
"""CensusLoss Trainium2 kernel.

Census transform loss: grayscale -> 48 shifted binary comparisons (7x7 patch,
reflect pad 3) -> mean |pred_census - target_census|.

Sharding: pure data parallel, batch dim B=8 across 8 NeuronCores (1 image per
core). Each core computes its local census mismatch count (an exact integer in
f32); host sums the per-partition partials and divides.

Per-core algorithm (all elementwise work on the Vector engine, bf16):
  1. gray = 0.299R + 0.587G + 0.114B  (f32, ACT mul + 2 fused scalar_tensor_tensor)
  2. build reflect-padded gray image [518 x 520] bf16 in DRAM (row width 520
     keeps every row 4B aligned in bf16 so DVE 2x_1P mode applies)
  3. load overlapping "band" layout: partition p holds padded rows 4p..4p+9
     flattened ([128, 5200]); a second copy shifted by one element (bandB)
     keeps odd-column-offset neighbor reads 4B-aligned
  4. per offset (di,dj): cmpP = is_gt(center, neighborP); cmpT likewise;
     tensor_tensor_reduce(cmpP != cmpT, add) accumulates the mismatch count
     into a per-partition f32 accumulator chained across all 48 offsets.

Comparisons are done in bf16: rounding f32->bf16 is monotonic, so only
near-ties (|delta| < ~2^-9 relative) can flip a comparison; with random
uniform inputs the net effect on the mean is ~1e-5 relative.
"""

import numpy as np

B, C, H, W = 8, 3, 512, 512
N_CORES = 8
PAD = 3
N_OFF = 48
Wp = 520            # padded row width (518 used + 2 spare, even for alignment)
Hp = 518            # padded rows
COL0 = 4            # padded col of gray col 0 (even => 4B-aligned in bf16)
RPP = 4             # gray rows per partition (512 / 128)
BAND_ROWS = RPP + 2 * PAD            # 10
BAND_LEN = BAND_ROWS * Wp            # 5200
ROW_TILE = RPP * Wp                  # 2080
FREE = RPP * W                       # 2048

_CACHE = {}


def _offsets():
    # even-dj offsets first: they only need the bandA loads, so the main loop
    # starts while the bandB (shifted) DMAs are still in flight
    evens, odds = [], []
    for di in range(-PAD, PAD + 1):
        for dj in range(-PAD, PAD + 1):
            if di == 0 and dj == 0:
                continue
            (evens if dj % 2 == 0 else odds).append((di, dj))
    return evens + odds


def _build_bass(n_off=N_OFF, repeat=1):
    from concourse import bacc, mybir
    from concourse.ap import AP
    from concourse.tile import TileContext
    from concourse.alu_op_type import AluOpType as op

    dt = mybir.dt
    # Bacc (not raw Bass): its compile() pass splits multi-sem waits into
    # event-semaphore NOPs — TRN2 instructions allow at most one wait each.
    nc = bacc.Bacc("TRN2", debug=False)

    pred = nc.dram_tensor("pred", [C, H, W], dt.float32, kind="ExternalInput")
    target = nc.dram_tensor("target", [C, H, W], dt.float32, kind="ExternalInput")
    # outputs: per-offset per-partition sums of cmpP (ACT), accumulated
    # column-sums of cmpT (PE ones-matmul), and the PSUM gram blocks whose
    # diagonal holds sum(cmpP*cmpT); host combines them exactly
    acc48_out = nc.dram_tensor("acc48_out", [128, max(n_off, 1)], dt.float32,
                               kind="ExternalOutput")
    sums_out = nc.dram_tensor("sums_out", [1, 512], dt.float32,
                              kind="ExternalOutput")
    prod_out = nc.dram_tensor("prod_out", [128, 128], dt.float32,
                              kind="ExternalOutput")
    pad_dram = {
        "p": nc.dram_tensor("pad_p", [Hp * Wp + 8], dt.bfloat16, kind="Internal"),
        "t": nc.dram_tensor("pad_t", [Hp * Wp + 8], dt.bfloat16, kind="Internal"),
    }

    with TileContext(nc) as tc:
        with tc.tile_pool(name="sbuf", bufs=1) as pool:
          for _rep in range(repeat):
            bands = {}
            for nm in ("p", "t"):
                for ab in ("A", "B"):
                    bands[nm + ab] = pool.tile(
                        [128, BAND_LEN], dt.bfloat16,
                        name=f"band_{nm}{ab}", tag=f"band_{nm}{ab}",
                    )

            # ---- grayscale + reflect-padded image, per tensor ----
            for nm, src in (("p", pred), ("t", target)):
                # two HWDGE queues (SP + ACT-seq) so pred/target transfers
                # run on different DMA rings
                qeng = nc.sync if nm == "p" else nc.scalar
                ch = []
                for c in range(3):
                    cht = pool.tile([128, FREE], dt.float32,
                                    name=f"ch_{nm}{c}", tag=f"ch{c}", bufs=1)
                    qeng.dma_start(
                        out=cht,
                        in_=src.ap()[c].rearrange("(p r) w -> p (r w)", p=128),
                    )
                    ch.append(cht)
                g1 = pool.tile([128, FREE], dt.bfloat16, name=f"g1_{nm}",
                               tag="g1", bufs=1)
                nc.scalar.mul(g1, ch[0], 0.299)
                gb = pool.tile([128, FREE], dt.bfloat16, name=f"gb_{nm}",
                               tag="gb", bufs=1)
                nc.scalar.mul(gb, ch[1], 0.587)
                gc = pool.tile([128, FREE], dt.bfloat16, name=f"gc_{nm}",
                               tag="gc", bufs=1)
                nc.scalar.mul(gc, ch[2], 0.114)
                g2 = pool.tile([128, FREE], dt.bfloat16, name=f"g2_{nm}",
                               tag="g2", bufs=1)
                nc.vector.tensor_add(g2, g1, gb)
                g3 = pool.tile([128, FREE], dt.bfloat16, name=f"g3_{nm}",
                               tag="g3", bufs=1)
                nc.vector.tensor_add(g3, g2, gc)

                g3v = g3.rearrange("p (r w) -> p r w", w=W)
                padt = pool.tile([128, ROW_TILE], dt.bfloat16,
                                 name=f"padt_{nm}", tag="padt", bufs=1)
                padv = padt.rearrange("p (r w) -> p r w", w=Wp)
                # zero the 2 spare cols (0 and 519) so DMA'd bytes are defined
                nc.vector.memset(
                    AP(padt.tensor, padt.offset, [[ROW_TILE, 128], [Wp, RPP], [Wp - 1, 2]]),
                    0.0)
                # center cols: gray col w -> padded col w+COL0 (casts f32->bf16)
                nc.vector.tensor_copy(out=padv[:, :, COL0:COL0 + W], in_=g3v)
                # reflect cols: padded col COL0-t = gray col t (t=1..3)
                nc.vector.tensor_copy(out=padv[:, :, 1:4], in_=g3v[:, :, 3:0:-1])
                # padded col COL0+W-1+t = gray col W-1-t
                nc.vector.tensor_copy(out=padv[:, :, 516:519], in_=g3v[:, :, 510:507:-1])

                # center rows 3..514 of the padded DRAM image
                qeng.dma_start(
                    out=AP(pad_dram[nm], PAD * Wp, [[ROW_TILE, 128], [1, ROW_TILE]]),
                    in_=padt)
                # reflect rows: padded row 0,1,2 = gray rows 3,2,1 (partition 0
                # slots 3,2,1); padded row 515,516,517 = gray rows 510,509,508
                # (partition 127 slots 2,1,0)
                for dst_row, part, slot in ((0, 0, 3), (1, 0, 2), (2, 0, 1),
                                            (515, 127, 2), (516, 127, 1), (517, 127, 0)):
                    qeng.dma_start(
                        out=pad_dram[nm].ap()[dst_row * Wp:(dst_row + 1) * Wp],
                        in_=AP(padt.tensor, padt.offset + part * ROW_TILE + slot * Wp,
                               [[ROW_TILE, 1], [1, Wp]]))

                # initialize the 8 slack elements past the padded image (bandB's
                # last partition reads one of them; content is never used)
                qeng.dma_start(
                    out=pad_dram[nm].ap()[Hp * Wp:Hp * Wp + 8],
                    in_=AP(padt.tensor, padt.offset, [[ROW_TILE, 1], [1, 8]]))

            # band loads: partition p <- padded rows 4p..4p+9 flattened; both
            # A bands load first so even-dj offsets start while the shifted
            # B bands are still in flight
            for ab, shift in (("A", 0), ("B", 1)):
                for nm, qeng in (("p", nc.sync), ("t", nc.scalar)):
                    qeng.dma_start(
                        out=bands[nm + ab],
                        in_=AP(pad_dram[nm], shift, [[RPP * Wp, 128], [1, BAND_LEN]]))

            # ---- main loop: 48 offsets ----
            centers = {
                nm: bands[nm + "A"].rearrange("p (r w) -> p r w", w=Wp)[
                    :, PAD:PAD + RPP, COL0:COL0 + W]
                for nm in ("p", "t")
            }

            # sum(xor) = sum(cmpP) + sum(cmpT) - 2*sum(cmpP*cmpT):
            #   DVE computes only the two is_gt maps per offset (bf16 2x mode);
            #   ACT sums cmpP via activation(Copy) accum_out;
            #   PE sums cmpT (ones-matmul) and accumulates the cmpP*cmpT gram
            #   blocks into PSUM -- only the diagonal is meaningful.
            acc48 = pool.tile([128, max(n_off, 1)], dt.float32,
                              name="acc48", tag="acc48")
            nc.vector.memset(acc48, 0.0)
            ones = pool.tile([128, 1], dt.bfloat16, name="ones", tag="ones")
            nc.vector.memset(ones, 1.0)
            with tc.tile_pool(name="psum", bufs=1, space="PSUM") as ppool:
                prod = ppool.tile([128, 128], dt.float32, name="prod")
                sums = ppool.tile([1, 512], dt.float32, name="sums")
                offs = _offsets()[:n_off]
                # a few offsets' comparisons can run on GPSIMD to relieve DVE
                gp_n = int(_CACHE.get("gpsimd_offsets", 0))
                gp_idx = {round((k + 0.5) * len(offs) / gp_n) for k in range(gp_n)} if gp_n else set()
                # every 8th offset's cmpP sum goes to PE instead of ACT
                pe_sum_idx = {i for i in range(len(offs)) if i % 8 == 7}
                for i, (di, dj) in enumerate(offs):
                    cmps = {}
                    eng = nc.gpsimd if i in gp_idx else nc.vector
                    for nm in ("p", "t"):
                        if dj % 2 == 0:
                            nb = bands[nm + "A"].rearrange("p (r w) -> p r w", w=Wp)[
                                :, PAD + di:PAD + di + RPP, COL0 + dj:COL0 + dj + W]
                        else:
                            nb = bands[nm + "B"].rearrange("p (r w) -> p r w", w=Wp)[
                                :, PAD + di:PAD + di + RPP,
                                COL0 + dj - 1:COL0 + dj - 1 + W]
                        cmp = pool.tile([128, FREE], dt.bfloat16,
                                        name=f"cmp_{nm}_{i}", tag=f"cmp_{nm}", bufs=4)
                        eng.tensor_tensor(
                            out=cmp.rearrange("p (r w) -> p r w", w=W),
                            in0=centers[nm], in1=nb, op=op.is_gt)
                        cmps[nm] = cmp
                    if i in pe_sum_idx:
                        for c in range(FREE // 512):
                            nc.tensor.matmul(
                                sums[0:1, :],
                                ones[:, 0:1],
                                cmps["p"][:, c * 512:(c + 1) * 512],
                                start=False, stop=False,
                                skip_group_check=True)
                    else:
                        dact = pool.tile([128, FREE], dt.bfloat16,
                                         name=f"dact_{i}", tag="dact", bufs=1)
                        nc.scalar.activation(
                            out=dact, in_=cmps["p"],
                            func=mybir.ActivationFunctionType.Copy,
                            accum_out=acc48[:, i:i + 1])
                    for c in range(FREE // 128):
                        nc.tensor.matmul(
                            prod[:, :],
                            cmps["p"][:, c * 128:(c + 1) * 128],
                            cmps["t"][:, c * 128:(c + 1) * 128],
                            start=(i == 0 and c == 0),
                            stop=(i == len(offs) - 1 and c == FREE // 128 - 1),
                            skip_group_check=True)
                    for c in range(FREE // 512):
                        nc.tensor.matmul(
                            sums[0:1, :],
                            ones[:, 0:1],
                            cmps["t"][:, c * 512:(c + 1) * 512],
                            start=(i == 0 and c == 0),
                            stop=(i == len(offs) - 1 and c == FREE // 512 - 1),
                            skip_group_check=True)

                prod_sb = pool.tile([128, 128], dt.float32, name="prod_sb",
                                    tag="prod_sb")
                sums_sb = pool.tile([1, 512], dt.float32, name="sums_sb",
                                    tag="sums_sb")
                if n_off == 0:
                    nc.vector.memset(acc48, 0.0)
                    nc.vector.memset(prod_sb, 0.0)
                    nc.vector.memset(sums_sb, 0.0)
                else:
                    nc.vector.tensor_copy(out=prod_sb, in_=prod)
                    nc.vector.tensor_copy(out=sums_sb, in_=sums)
                nc.sync.dma_start(out=acc48_out.ap(), in_=acc48)
                nc.sync.dma_start(out=prod_out.ap(), in_=prod_sb)
                nc.sync.dma_start(out=sums_out.ap(), in_=sums_sb)

    nc.finalize()
    return nc


def kernel(pred: np.ndarray, target: np.ndarray) -> np.ndarray:
    from concourse import bass_utils

    if "nc" not in _CACHE:
        _CACHE["nc"] = _build_bass()
    nc = _CACHE["nc"]

    pred = np.ascontiguousarray(pred, dtype=np.float32)
    target = np.ascontiguousarray(target, dtype=np.float32)
    in_maps = [
        {"pred": pred[b], "target": target[b]} for b in range(N_CORES)
    ]
    res = bass_utils.run_bass_kernel_spmd(nc, in_maps, core_ids=list(range(N_CORES)))
    total = 0.0
    for r in res.results:
        total += float(r["acc48_out"].astype(np.float64).sum())
        total += float(r["sums_out"].astype(np.float64).sum())
        total -= 2.0 * float(np.diag(r["prod_out"]).astype(np.float64).sum())
    mean = total / (B * N_OFF * H * W)
    return np.float32(mean)


# revision 24
# speedup vs baseline: 1.0783x; 1.0783x over previous
"""CensusLoss Trainium2 kernel.

Census transform loss: grayscale -> 48 shifted binary comparisons (7x7 patch,
reflect pad 3) -> mean |pred_census - target_census|.

Sharding: pure data parallel, batch dim B=8 across 8 NeuronCores (one image
per core). Each core emits exact integer partial sums (in f32); the host
combines them and divides.

Per-core pipeline:
  1. gray = 0.299R + 0.587G + 0.114B (ACT muls -> bf16, DVE adds), written
     into a column-reflect-padded row tile `padt` [128 part x 4 rows x 520]
     (row width 520 keeps every bf16 row 4B-aligned => DVE 2x_1P mode).
  2. "band" layout via SBUF->SBUF DMAs only: partition p holds padded rows
     4p..4p+9 flattened ([128, 5200]) — center rows from padt[p], halo rows
     from padt[p-1]/padt[p+1] (partition-shifted affine DMAs), reflect rows
     at the image edges from per-row copies. bandB = bandA shifted one
     element (keeps odd-column-offset neighbor reads 4B-aligned).
  3. Per offset (di,dj): cmpP = is_gt(center, neighbor), cmpT likewise — the
     only per-offset DVE work (bf16 2x mode, ~1us per [128,2048] op).
     sum(xor) = sum(cmpP) + sum(cmpT) - 2*sum(cmpP*cmpT):
       - sum(cmpP): ACT activation(Copy) with accum_out (idle engine)
       - sum(cmpT): PE ones-matmul accumulated in PSUM
       - sum(cmpP*cmpT): PE gram blocks accumulated in PSUM; only the
         diagonal of the [128,128] result is meaningful.
  4. Host: total = sum(acc48) + sum(sums) - 2*trace(prod), exact integers.

Comparisons run in bf16: f32->bf16 rounding is monotonic, so only near-ties
can flip a comparison; measured effect on the mean is ~2e-6 relative.
"""

import numpy as np

B, C, H, W = 8, 3, 512, 512
N_CORES = 8
PAD = 3
N_OFF = 48
Wp = 520            # padded row width (518 used + 2 spare, even for alignment)
COL0 = 4            # padded col of gray col 0 (even => 4B-aligned in bf16)
RPP = 4             # gray rows per partition (512 / 128)
BAND_ROWS = RPP + 2 * PAD            # 10
BAND_LEN = BAND_ROWS * Wp            # 5200
ROW_TILE = RPP * Wp                  # 2080
FREE = RPP * W                       # 2048

_CACHE = {}


def _offsets():
    # even-dj offsets first: they only need the bandA construction, so the
    # main loop starts while the shifted bandB copies are still in flight
    evens, odds = [], []
    for di in range(-PAD, PAD + 1):
        for dj in range(-PAD, PAD + 1):
            if di == 0 and dj == 0:
                continue
            (evens if dj % 2 == 0 else odds).append((di, dj))
    return evens + odds


def _build_bass(n_off=N_OFF, repeat=1):
    from concourse import bacc, mybir
    from concourse.ap import AP
    from concourse.tile import TileContext
    from concourse.alu_op_type import AluOpType as op

    dt = mybir.dt
    # Bacc (not raw Bass): its compile() pass splits multi-sem waits into
    # event-semaphore NOPs — TRN2 instructions allow at most one wait each.
    nc = bacc.Bacc("TRN2", debug=False)

    pred = nc.dram_tensor("pred", [C, H, W], dt.float32, kind="ExternalInput")
    target = nc.dram_tensor("target", [C, H, W], dt.float32, kind="ExternalInput")
    acc48_out = nc.dram_tensor("acc48_out", [128, max(n_off, 1)], dt.float32,
                               kind="ExternalOutput")
    sums_out = nc.dram_tensor("sums_out", [1, 512], dt.float32,
                              kind="ExternalOutput")
    prod_out = nc.dram_tensor("prod_out", [128, 128], dt.float32,
                              kind="ExternalOutput")

    def band_view(t, r0, c0):
        # [128, RPP rows, W cols] view of a band tile at row r0, col c0
        return t.rearrange("p (r w) -> p r w", w=Wp)[
            :, r0:r0 + RPP, c0:c0 + W]

    with TileContext(nc) as tc:
      with tc.tile_pool(name="sbuf", bufs=1) as pool:
        for _rep in range(repeat):
            bands = {}
            for nm in ("p", "t"):
                for ab in ("A", "B"):
                    bands[nm + ab] = pool.tile(
                        [128, BAND_LEN], dt.bfloat16,
                        name=f"band_{nm}{ab}", tag=f"band_{nm}{ab}",
                    )

            padts = {}
            for nm, src in (("p", pred), ("t", target)):
                # two HWDGE queues (SP + ACT-seq) so pred/target transfers
                # use different DMA rings
                qeng = nc.sync if nm == "p" else nc.scalar
                ch = []
                for c in range(3):
                    cht = pool.tile([128, FREE], dt.float32,
                                    name=f"ch_{nm}{c}", tag=f"ch{c}", bufs=1)
                    qeng.dma_start(
                        out=cht,
                        in_=src.ap()[c].rearrange("(p r) w -> p (r w)", p=128),
                    )
                    ch.append(cht)
                g1 = pool.tile([128, FREE], dt.bfloat16, name=f"g1_{nm}",
                               tag="g1", bufs=1)
                nc.scalar.mul(g1, ch[0], 0.299)
                gb = pool.tile([128, FREE], dt.bfloat16, name=f"gb_{nm}",
                               tag="gb", bufs=1)
                nc.scalar.mul(gb, ch[1], 0.587)
                gc = pool.tile([128, FREE], dt.bfloat16, name=f"gc_{nm}",
                               tag="gc", bufs=1)
                nc.scalar.mul(gc, ch[2], 0.114)
                g2 = pool.tile([128, FREE], dt.bfloat16, name=f"g2_{nm}",
                               tag="g2", bufs=1)
                nc.vector.tensor_add(g2, g1, gb)
                g3 = pool.tile([128, FREE], dt.bfloat16, name=f"g3_{nm}",
                               tag="g3", bufs=1)
                nc.vector.tensor_add(g3, g2, gc)

                g3v = g3.rearrange("p (r w) -> p r w", w=W)
                padt = pool.tile([128, ROW_TILE], dt.bfloat16,
                                 name=f"padt_{nm}", tag=f"padt_{nm}", bufs=1)
                padts[nm] = padt
                padv = padt.rearrange("p (r w) -> p r w", w=Wp)
                # zero the 2 spare cols (0 and 519) so band DMAs carry
                # defined bytes
                nc.vector.memset(
                    AP(padt.tensor, padt.offset,
                       [[ROW_TILE, 128], [Wp, RPP], [Wp - 1, 2]]),
                    0.0)
                # center cols: gray col w -> padded col w+COL0
                nc.vector.tensor_copy(out=padv[:, :, COL0:COL0 + W], in_=g3v)
                # reflect cols: padded col COL0-t = gray col t (t=1..3)
                nc.vector.tensor_copy(out=padv[:, :, 1:4], in_=g3v[:, :, 3:0:-1])
                # padded col COL0+W-1+t = gray col W-1-t
                nc.vector.tensor_copy(out=padv[:, :, 516:519],
                                      in_=g3v[:, :, 510:507:-1])

            # ---- band construction, all SBUF->SBUF ----
            for nm in ("p", "t"):
                qeng = nc.sync if nm == "p" else nc.scalar
                padt = padts[nm]
                bA = bands[nm + "A"]
                pstride_p = padt.ap[0][0]
                pstride_b = bA.ap[0][0]
                # center rows: bandA[p][slots 3..6] <- padt[p][slots 0..3]
                qeng.dma_start(
                    out=AP(bA.tensor, bA.offset + 3 * Wp,
                           [[pstride_b, 128], [1, ROW_TILE]]),
                    in_=padt)
                # top halo: bandA[p][slots 0..2] <- padt[p-1][slots 1..3]
                qeng.dma_start(
                    out=AP(bA.tensor, bA.offset + 1 * pstride_b,
                           [[pstride_b, 127], [1, 3 * Wp]]),
                    in_=AP(padt.tensor, padt.offset + 1 * Wp,
                           [[pstride_p, 127], [1, 3 * Wp]]))
                # bottom halo: bandA[p][slots 7..9] <- padt[p+1][slots 0..2]
                qeng.dma_start(
                    out=AP(bA.tensor, bA.offset + 7 * Wp,
                           [[pstride_b, 127], [1, 3 * Wp]]),
                    in_=AP(padt.tensor, padt.offset + 1 * pstride_p,
                           [[pstride_p, 127], [1, 3 * Wp]]))
                # reflect edges: partition 0 top = gray rows 3,2,1
                # (padt[0] slots 3,2,1); partition 127 bottom = gray rows
                # 510,509,508 (padt[127] slots 2,1,0)
                for s_band, slot in ((0, 3), (1, 2), (2, 1)):
                    qeng.dma_start(
                        out=AP(bA.tensor, bA.offset + s_band * Wp,
                               [[pstride_b, 1], [1, Wp]]),
                        in_=AP(padt.tensor, padt.offset + slot * Wp,
                               [[pstride_p, 1], [1, Wp]]))
                for s_band, slot in ((7, 2), (8, 1), (9, 0)):
                    qeng.dma_start(
                        out=AP(bA.tensor,
                               bA.offset + 127 * pstride_b + s_band * Wp,
                               [[pstride_b, 1], [1, Wp]]),
                        in_=AP(padt.tensor,
                               padt.offset + 127 * pstride_p + slot * Wp,
                               [[pstride_p, 1], [1, Wp]]))
            # bandB = bandA shifted left one element (last element unused and
            # never read by any compute view)
            for nm in ("p", "t"):
                qeng = nc.sync if nm == "p" else nc.scalar
                bA, bB = bands[nm + "A"], bands[nm + "B"]
                qeng.dma_start(out=bB[:, 0:BAND_LEN - 1],
                               in_=bA[:, 1:BAND_LEN])

            # ---- main loop ----
            centers = {nm: band_view(bands[nm + "A"], PAD, COL0)
                       for nm in ("p", "t")}
            acc48 = pool.tile([128, max(n_off, 1)], dt.float32,
                              name="acc48", tag="acc48")
            nc.vector.memset(acc48, 0.0)
            ones = pool.tile([128, 1], dt.bfloat16, name="ones", tag="ones")
            nc.vector.memset(ones, 1.0)
            with tc.tile_pool(name="psum", bufs=1, space="PSUM") as ppool:
                prod = ppool.tile([128, 128], dt.float32, name="prod")
                sums = ppool.tile([1, 512], dt.float32, name="sums")
                offs = _offsets()[:n_off]
                # every 8th offset's cmpP sum goes to PE instead of ACT
                pe_sum_idx = {i for i in range(len(offs)) if i % 8 == 7}
                for i, (di, dj) in enumerate(offs):
                    cmps = {}
                    for nm in ("p", "t"):
                        if dj % 2 == 0:
                            nb = band_view(bands[nm + "A"], PAD + di, COL0 + dj)
                        else:
                            nb = band_view(bands[nm + "B"], PAD + di,
                                           COL0 + dj - 1)
                        cmp = pool.tile([128, FREE], dt.bfloat16,
                                        name=f"cmp_{nm}_{i}", tag=f"cmp_{nm}",
                                        bufs=10)
                        nc.vector.tensor_tensor(
                            out=cmp.rearrange("p (r w) -> p r w", w=W),
                            in0=centers[nm], in1=nb, op=op.is_gt)
                        cmps[nm] = cmp
                    if i in pe_sum_idx:
                        for c in range(FREE // 512):
                            nc.tensor.matmul(
                                sums[0:1, :], ones[:, 0:1],
                                cmps["p"][:, c * 512:(c + 1) * 512],
                                start=False, stop=False,
                                skip_group_check=True)
                    else:
                        dact = pool.tile([128, FREE], dt.bfloat16,
                                         name=f"dact_{i}", tag="dact", bufs=1)
                        nc.scalar.activation(
                            out=dact, in_=cmps["p"],
                            func=mybir.ActivationFunctionType.Copy,
                            accum_out=acc48[:, i:i + 1])
                    for c in range(FREE // 128):
                        nc.tensor.matmul(
                            prod[:, :],
                            cmps["p"][:, c * 128:(c + 1) * 128],
                            cmps["t"][:, c * 128:(c + 1) * 128],
                            start=(i == 0 and c == 0),
                            stop=(i == len(offs) - 1 and c == FREE // 128 - 1),
                            skip_group_check=True)
                    for c in range(FREE // 512):
                        nc.tensor.matmul(
                            sums[0:1, :], ones[:, 0:1],
                            cmps["t"][:, c * 512:(c + 1) * 512],
                            start=(i == 0 and c == 0),
                            stop=(i == len(offs) - 1 and c == FREE // 512 - 1),
                            skip_group_check=True)

                prod_sb = pool.tile([128, 128], dt.float32, name="prod_sb",
                                    tag="prod_sb")
                sums_sb = pool.tile([1, 512], dt.float32, name="sums_sb",
                                    tag="sums_sb")
                if n_off == 0:
                    nc.vector.memset(prod_sb, 0.0)
                    nc.vector.memset(sums_sb, 0.0)
                else:
                    nc.vector.tensor_copy(out=prod_sb, in_=prod)
                    nc.vector.tensor_copy(out=sums_sb, in_=sums)
                nc.sync.dma_start(out=acc48_out.ap(), in_=acc48)
                nc.sync.dma_start(out=prod_out.ap(), in_=prod_sb)
                nc.sync.dma_start(out=sums_out.ap(), in_=sums_sb)

    nc.finalize()
    return nc


def kernel(pred: np.ndarray, target: np.ndarray) -> np.ndarray:
    from concourse import bass_utils

    if "nc" not in _CACHE:
        _CACHE["nc"] = _build_bass()
    nc = _CACHE["nc"]

    pred = np.ascontiguousarray(pred, dtype=np.float32)
    target = np.ascontiguousarray(target, dtype=np.float32)
    in_maps = [
        {"pred": pred[b], "target": target[b]} for b in range(N_CORES)
    ]
    res = bass_utils.run_bass_kernel_spmd(nc, in_maps,
                                          core_ids=list(range(N_CORES)))
    total = 0.0
    for r in res.results:
        total += float(r["acc48_out"].astype(np.float64).sum())
        total += float(r["sums_out"].astype(np.float64).sum())
        total -= 2.0 * float(np.diag(r["prod_out"]).astype(np.float64).sum())
    mean = total / (B * N_OFF * H * W)
    return np.array(mean, dtype=np.float32)


# revision 27
# speedup vs baseline: 1.0963x; 1.0167x over previous
"""CensusLoss Trainium2 kernel.

Census transform loss: grayscale -> 48 shifted binary comparisons (7x7 patch,
reflect pad 3) -> mean |pred_census - target_census|.

Sharding: pure data parallel, batch dim B=8 across 8 NeuronCores (one image
per core). Each core emits exact integer partial sums (in f32); the host
combines them and divides.

Per-core pipeline:
  1. gray = 0.299R + 0.587G + 0.114B (ACT muls -> bf16, DVE adds), written
     column-reflect-padded DIRECTLY into the center rows of the "band" tile
     (row width 520 keeps every bf16 row 4B-aligned => DVE 2x_1P mode).
  2. band layout: partition p holds padded rows 4p..4p+9 flattened
     ([128, 5200]); only the 3+3 halo rows need DMAs (partition-shifted
     SBUF->SBUF affine copies from the neighbors' center rows), plus per-row
     reflect copies at the image edges. bandB = bandA shifted one element
     (keeps odd-column-offset neighbor reads 4B-aligned).
  3. Per offset (di,dj): cmpP = is_gt(center, neighbor), cmpT likewise — the
     only per-offset DVE work (bf16 2x mode, ~1us per [128,2048] op).
     sum(xor) = sum(cmpP) + sum(cmpT) - 2*sum(cmpP*cmpT):
       - sum(cmpP): ACT activation(Copy) with accum_out (idle engine)
       - sum(cmpT): PE ones-matmul accumulated in PSUM
       - sum(cmpP*cmpT): PE gram blocks accumulated in PSUM; only the
         diagonal of the [128,128] result is meaningful.
  4. Host: total = sum(acc48) + sum(sums) - 2*trace(prod), exact integers.

Comparisons run in bf16: f32->bf16 rounding is monotonic, so only near-ties
can flip a comparison; measured effect on the mean is ~2e-6 relative.
"""

import numpy as np

B, C, H, W = 8, 3, 512, 512
N_CORES = 8
PAD = 3
N_OFF = 48
Wp = 520            # padded row width (518 used + 2 spare, even for alignment)
COL0 = 4            # padded col of gray col 0 (even => 4B-aligned in bf16)
RPP = 4             # gray rows per partition (512 / 128)
BAND_ROWS = RPP + 2 * PAD            # 10
BAND_LEN = BAND_ROWS * Wp            # 5200
ROW_TILE = RPP * Wp                  # 2080
FREE = RPP * W                       # 2048

_CACHE = {}


def _offsets():
    # even-dj offsets first: they only need the bandA construction, so the
    # main loop starts while the shifted bandB copies are still in flight
    evens, odds = [], []
    for di in range(-PAD, PAD + 1):
        for dj in range(-PAD, PAD + 1):
            if di == 0 and dj == 0:
                continue
            (evens if dj % 2 == 0 else odds).append((di, dj))
    return evens + odds


def _build_bass(n_off=N_OFF, repeat=1):
    from concourse import bacc, mybir
    from concourse.ap import AP
    from concourse.tile import TileContext
    from concourse.alu_op_type import AluOpType as op

    dt = mybir.dt
    # Bacc (not raw Bass): its compile() pass splits multi-sem waits into
    # event-semaphore NOPs — TRN2 instructions allow at most one wait each.
    nc = bacc.Bacc("TRN2", debug=False)

    pred = nc.dram_tensor("pred", [C, H, W], dt.float32, kind="ExternalInput")
    target = nc.dram_tensor("target", [C, H, W], dt.float32, kind="ExternalInput")
    acc48_out = nc.dram_tensor("acc48_out", [128, max(n_off, 1)], dt.float32,
                               kind="ExternalOutput")
    sums_out = nc.dram_tensor("sums_out", [1, 512], dt.float32,
                              kind="ExternalOutput")
    prod_out = nc.dram_tensor("prod_out", [128, 128], dt.float32,
                              kind="ExternalOutput")

    def band_view(t, r0, c0):
        # [128, RPP rows, W cols] view of a band tile at row r0, col c0
        return t.rearrange("p (r w) -> p r w", w=Wp)[
            :, r0:r0 + RPP, c0:c0 + W]

    with TileContext(nc) as tc:
      with tc.tile_pool(name="sbuf", bufs=1) as pool:
        for _rep in range(repeat):
            bands = {}
            for nm in ("p", "t"):
                for ab in ("A", "B"):
                    bands[nm + ab] = pool.tile(
                        [128, BAND_LEN], dt.bfloat16,
                        name=f"band_{nm}{ab}", tag=f"band_{nm}{ab}",
                    )

            # channel loads interleaved across the two HWDGE queues (SP +
            # ACT-seq) with pred's channels at the FRONT of both queues:
            # pred finishes first so its gray/band build overlaps target's
            # remaining input transfers
            chs = {}
            load_order = [("p", 0, nc.sync), ("p", 1, nc.scalar),
                          ("p", 2, nc.sync), ("t", 0, nc.scalar),
                          ("t", 1, nc.sync), ("t", 2, nc.scalar)]
            for nm, c, q in load_order:
                src = pred if nm == "p" else target
                cht = pool.tile([128, FREE], dt.float32,
                                name=f"ch_{nm}{c}", tag=f"ch_{nm}{c}", bufs=1)
                q.dma_start(
                    out=cht,
                    in_=src.ap()[c].rearrange("(p r) w -> p (r w)", p=128),
                )
                chs[(nm, c)] = cht

            for nm, src in (("p", pred), ("t", target)):
                qeng = nc.sync if nm == "p" else nc.scalar
                ch = [chs[(nm, c)] for c in range(3)]
                g1 = pool.tile([128, FREE], dt.bfloat16, name=f"g1_{nm}",
                               tag="g1", bufs=1)
                nc.scalar.mul(g1, ch[0], 0.299)
                gb = pool.tile([128, FREE], dt.bfloat16, name=f"gb_{nm}",
                               tag="gb", bufs=1)
                nc.scalar.mul(gb, ch[1], 0.587)
                gc = pool.tile([128, FREE], dt.bfloat16, name=f"gc_{nm}",
                               tag="gc", bufs=1)
                nc.scalar.mul(gc, ch[2], 0.114)
                g2 = pool.tile([128, FREE], dt.bfloat16, name=f"g2_{nm}",
                               tag="g2", bufs=1)
                nc.vector.tensor_add(g2, g1, gb)
                g3 = pool.tile([128, FREE], dt.bfloat16, name=f"g3_{nm}",
                               tag="g3", bufs=1)
                nc.vector.tensor_add(g3, g2, gc)

                g3v = g3.rearrange("p (r w) -> p r w", w=W)
                # gray rows are written straight into the band tile's center
                # slots (rows 3..6): bandA then only needs the halo DMAs
                bA = bands[nm + "A"]
                padv = bA.rearrange("p (r w) -> p r w", w=Wp)[:, PAD:PAD + RPP, :]
                # zero the 2 spare cols (0 and 519) so halo DMAs carry
                # defined bytes
                nc.vector.memset(
                    AP(bA.tensor, bA.offset + PAD * Wp,
                       [[BAND_LEN, 128], [Wp, RPP], [Wp - 1, 2]]),
                    0.0)
                # center cols: gray col w -> padded col w+COL0
                nc.vector.tensor_copy(out=padv[:, :, COL0:COL0 + W], in_=g3v)
                # reflect cols: padded col COL0-t = gray col t (t=1..3)
                nc.vector.tensor_copy(out=padv[:, :, 1:4], in_=g3v[:, :, 3:0:-1])
                # padded col COL0+W-1+t = gray col W-1-t
                nc.vector.tensor_copy(out=padv[:, :, 516:519],
                                      in_=g3v[:, :, 510:507:-1])

            # ---- halo construction, all SBUF->SBUF within the band ----
            # center slot s (gray row 4p+s) lives at band offset (3+s)*Wp
            for nm in ("t", "p"):
                qeng = nc.sync if nm == "p" else nc.scalar
                bA = bands[nm + "A"]
                pstride_b = bA.ap[0][0]
                # top halo: band[p][slots 0..2] <- band[p-1][center slots 1..3]
                qeng.dma_start(
                    out=AP(bA.tensor, bA.offset + 1 * pstride_b,
                           [[pstride_b, 127], [1, 3 * Wp]]),
                    in_=AP(bA.tensor, bA.offset + 4 * Wp,
                           [[pstride_b, 127], [1, 3 * Wp]]))
                # bottom halo: band[p][slots 7..9] <- band[p+1][center 0..2]
                qeng.dma_start(
                    out=AP(bA.tensor, bA.offset + 7 * Wp,
                           [[pstride_b, 127], [1, 3 * Wp]]),
                    in_=AP(bA.tensor, bA.offset + 1 * pstride_b + 3 * Wp,
                           [[pstride_b, 127], [1, 3 * Wp]]))
                # reflect edges: partition 0 top = gray rows 3,2,1 (center
                # slots 3,2,1); partition 127 bottom = gray rows 510,509,508
                # (center slots 2,1,0)
                for s_band, slot in ((0, 3), (1, 2), (2, 1)):
                    qeng.dma_start(
                        out=AP(bA.tensor, bA.offset + s_band * Wp,
                               [[pstride_b, 1], [1, Wp]]),
                        in_=AP(bA.tensor, bA.offset + (PAD + slot) * Wp,
                               [[pstride_b, 1], [1, Wp]]))
                for s_band, slot in ((7, 2), (8, 1), (9, 0)):
                    qeng.dma_start(
                        out=AP(bA.tensor,
                               bA.offset + 127 * pstride_b + s_band * Wp,
                               [[pstride_b, 1], [1, Wp]]),
                        in_=AP(bA.tensor,
                               bA.offset + 127 * pstride_b + (PAD + slot) * Wp,
                               [[pstride_b, 1], [1, Wp]]))
            # bandB = bandA shifted left one element (last element unused and
            # never read by any compute view)
            for nm in ("p", "t"):
                qeng = nc.sync if nm == "p" else nc.scalar
                bA, bB = bands[nm + "A"], bands[nm + "B"]
                qeng.dma_start(out=bB[:, 0:BAND_LEN - 1],
                               in_=bA[:, 1:BAND_LEN])

            # ---- main loop ----
            centers = {nm: band_view(bands[nm + "A"], PAD, COL0)
                       for nm in ("p", "t")}
            acc48 = pool.tile([128, max(n_off, 1)], dt.float32,
                              name="acc48", tag="acc48")
            nc.vector.memset(acc48, 0.0)
            ones = pool.tile([128, 1], dt.bfloat16, name="ones", tag="ones")
            nc.vector.memset(ones, 1.0)
            with tc.tile_pool(name="psum", bufs=1, space="PSUM") as ppool:
                prod = ppool.tile([128, 128], dt.float32, name="prod")
                sums = ppool.tile([1, 512], dt.float32, name="sums")
                offs = _offsets()[:n_off]
                # every 8th offset's cmpP sum goes to PE instead of ACT
                pe_sum_idx = {i for i in range(len(offs)) if i % 8 == 7}
                for i, (di, dj) in enumerate(offs):
                    cmps = {}
                    for nm in ("p", "t"):
                        if dj % 2 == 0:
                            nb = band_view(bands[nm + "A"], PAD + di, COL0 + dj)
                        else:
                            nb = band_view(bands[nm + "B"], PAD + di,
                                           COL0 + dj - 1)
                        cmp = pool.tile([128, FREE], dt.bfloat16,
                                        name=f"cmp_{nm}_{i}", tag=f"cmp_{nm}",
                                        bufs=10)
                        nc.vector.tensor_tensor(
                            out=cmp.rearrange("p (r w) -> p r w", w=W),
                            in0=centers[nm], in1=nb, op=op.is_gt)
                        cmps[nm] = cmp
                    if i in pe_sum_idx:
                        for c in range(FREE // 512):
                            nc.tensor.matmul(
                                sums[0:1, :], ones[:, 0:1],
                                cmps["p"][:, c * 512:(c + 1) * 512],
                                start=False, stop=False,
                                skip_group_check=True)
                    else:
                        dact = pool.tile([128, FREE], dt.bfloat16,
                                         name=f"dact_{i}", tag="dact", bufs=1)
                        nc.scalar.activation(
                            out=dact, in_=cmps["p"],
                            func=mybir.ActivationFunctionType.Copy,
                            accum_out=acc48[:, i:i + 1])
                    for c in range(FREE // 128):
                        nc.tensor.matmul(
                            prod[:, :],
                            cmps["p"][:, c * 128:(c + 1) * 128],
                            cmps["t"][:, c * 128:(c + 1) * 128],
                            start=(i == 0 and c == 0),
                            stop=(i == len(offs) - 1 and c == FREE // 128 - 1),
                            skip_group_check=True)
                    for c in range(FREE // 512):
                        nc.tensor.matmul(
                            sums[0:1, :], ones[:, 0:1],
                            cmps["t"][:, c * 512:(c + 1) * 512],
                            start=(i == 0 and c == 0),
                            stop=(i == len(offs) - 1 and c == FREE // 512 - 1),
                            skip_group_check=True)

                prod_sb = pool.tile([128, 128], dt.float32, name="prod_sb",
                                    tag="prod_sb")
                sums_sb = pool.tile([1, 512], dt.float32, name="sums_sb",
                                    tag="sums_sb")
                if n_off == 0:
                    nc.vector.memset(prod_sb, 0.0)
                    nc.vector.memset(sums_sb, 0.0)
                else:
                    nc.vector.tensor_copy(out=prod_sb, in_=prod)
                    nc.vector.tensor_copy(out=sums_sb, in_=sums)
                nc.sync.dma_start(out=acc48_out.ap(), in_=acc48)
                nc.sync.dma_start(out=prod_out.ap(), in_=prod_sb)
                nc.sync.dma_start(out=sums_out.ap(), in_=sums_sb)

    nc.finalize()
    return nc


def kernel(pred: np.ndarray, target: np.ndarray) -> np.ndarray:
    from concourse import bass_utils

    if "nc" not in _CACHE:
        _CACHE["nc"] = _build_bass()
    nc = _CACHE["nc"]

    pred = np.ascontiguousarray(pred, dtype=np.float32)
    target = np.ascontiguousarray(target, dtype=np.float32)
    in_maps = [
        {"pred": pred[b], "target": target[b]} for b in range(N_CORES)
    ]
    res = bass_utils.run_bass_kernel_spmd(nc, in_maps,
                                          core_ids=list(range(N_CORES)))
    total = 0.0
    for r in res.results:
        total += float(r["acc48_out"].astype(np.float64).sum())
        total += float(r["sums_out"].astype(np.float64).sum())
        total -= 2.0 * float(np.diag(r["prod_out"]).astype(np.float64).sum())
    mean = total / (B * N_OFF * H * W)
    return np.array(mean, dtype=np.float32)


# revision 31
# speedup vs baseline: 1.1534x; 1.0521x over previous
"""CensusLoss Trainium2 kernel.

Census transform loss: grayscale -> 48 shifted binary comparisons (7x7 patch,
reflect pad 3) -> mean |pred_census - target_census|.

Sharding: pure data parallel, batch dim B=8 across 8 NeuronCores (one image
per core). Each core emits exact integer partial sums (in f32); the host
combines them and divides.

Per-core pipeline:
  1. gray = 0.299R + 0.587G + 0.114B (ACT muls -> bf16, DVE adds), written
     column-reflect-padded DIRECTLY into the center rows of the "band" tile
     (row width 520 keeps every bf16 row 4B-aligned => DVE 2x_1P mode).
  2. band layout: partition p holds padded rows 4p..4p+9 flattened
     ([128, 5200]); only the 3+3 halo rows need DMAs (partition-shifted
     SBUF->SBUF affine copies from the neighbors' center rows), plus per-row
     reflect copies at the image edges. bandB = bandA shifted one element
     (keeps odd-column-offset neighbor reads 4B-aligned).
  3. Per offset (di,dj): cmpP = is_gt(center, neighbor), cmpT likewise
     (bf16 2x mode, ~1us per [128,2048] op). Every 6th offset instead
     computes d = center - neighbor on the otherwise-idle GPSIMD engine and
     binarizes on DVE with tensor_scalar(d > 0) in 4x mode (bf16 subtraction
     sign is exact, so results are identical).
     sum(xor) = sum(cmpP) + sum(cmpT) - 2*sum(cmpP*cmpT):
       - sum(cmpP): ACT activation(Copy) with accum_out (idle engine)
       - sum(cmpT): PE ones-matmul accumulated in PSUM
       - sum(cmpP*cmpT): PE gram blocks accumulated in PSUM; only the
         diagonal of the [128,128] result is meaningful.
  4. Host: total = sum(acc48) + sum(sums) - 2*trace(prod), exact integers.

Comparisons run in bf16: f32->bf16 rounding is monotonic, so only near-ties
can flip a comparison; measured effect on the mean is ~2e-6 relative.
"""

import numpy as np

B, C, H, W = 8, 3, 512, 512
N_CORES = 8
PAD = 3
N_OFF = 48
Wp = 520            # padded row width (518 used + 2 spare, even for alignment)
COL0 = 4            # padded col of gray col 0 (even => 4B-aligned in bf16)
RPP = 4             # gray rows per partition (512 / 128)
BAND_ROWS = RPP + 2 * PAD            # 10
BAND_LEN = BAND_ROWS * Wp            # 5200
ROW_TILE = RPP * Wp                  # 2080
FREE = RPP * W                       # 2048

_CACHE = {}


def _offsets():
    # even-dj offsets first: they only need the bandA construction, so the
    # main loop starts while the shifted bandB copies are still in flight
    evens, odds = [], []
    for di in range(-PAD, PAD + 1):
        for dj in range(-PAD, PAD + 1):
            if di == 0 and dj == 0:
                continue
            (evens if dj % 2 == 0 else odds).append((di, dj))
    return evens + odds


def _build_bass(n_off=N_OFF, repeat=1):
    from concourse import bacc, mybir
    from concourse.ap import AP
    from concourse.tile import TileContext
    from concourse.alu_op_type import AluOpType as op

    dt = mybir.dt
    # Bacc (not raw Bass): its compile() pass splits multi-sem waits into
    # event-semaphore NOPs — TRN2 instructions allow at most one wait each.
    nc = bacc.Bacc("TRN2", debug=False)

    pred = nc.dram_tensor("pred", [C, H, W], dt.float32, kind="ExternalInput")
    target = nc.dram_tensor("target", [C, H, W], dt.float32, kind="ExternalInput")
    acc48_out = nc.dram_tensor("acc48_out", [128, max(n_off, 1)], dt.float32,
                               kind="ExternalOutput")
    sums_out = nc.dram_tensor("sums_out", [1, 512], dt.float32,
                              kind="ExternalOutput")
    prod_out = nc.dram_tensor("prod_out", [128, 128], dt.float32,
                              kind="ExternalOutput")

    def band_view(t, r0, c0):
        # [128, RPP rows, W cols] view of a band tile at row r0, col c0
        return t.rearrange("p (r w) -> p r w", w=Wp)[
            :, r0:r0 + RPP, c0:c0 + W]

    with TileContext(nc) as tc:
      with tc.tile_pool(name="sbuf", bufs=1) as pool:
        for _rep in range(repeat):
            bands = {}
            for nm in ("p", "t"):
                for ab in ("A", "B"):
                    bands[nm + ab] = pool.tile(
                        [128, BAND_LEN], dt.bfloat16,
                        name=f"band_{nm}{ab}", tag=f"band_{nm}{ab}",
                    )

            # channel loads interleaved across the two HWDGE queues (SP +
            # ACT-seq) with pred's channels at the FRONT of both queues:
            # pred finishes first so its gray/band build overlaps target's
            # remaining input transfers
            chs = {}
            load_order = [("p", 0, nc.sync), ("p", 1, nc.scalar),
                          ("p", 2, nc.sync), ("t", 0, nc.scalar),
                          ("t", 1, nc.sync), ("t", 2, nc.scalar)]
            for nm, c, q in load_order:
                src = pred if nm == "p" else target
                cht = pool.tile([128, FREE], dt.float32,
                                name=f"ch_{nm}{c}", tag=f"ch_{nm}{c}", bufs=1)
                q.dma_start(
                    out=cht,
                    in_=src.ap()[c].rearrange("(p r) w -> p (r w)", p=128),
                )
                chs[(nm, c)] = cht

            for nm, src in (("p", pred), ("t", target)):
                qeng = nc.sync if nm == "p" else nc.scalar
                ch = [chs[(nm, c)] for c in range(3)]
                g1 = pool.tile([128, FREE], dt.bfloat16, name=f"g1_{nm}",
                               tag="g1", bufs=1)
                nc.scalar.mul(g1, ch[0], 0.299)
                gb = pool.tile([128, FREE], dt.bfloat16, name=f"gb_{nm}",
                               tag="gb", bufs=1)
                nc.scalar.mul(gb, ch[1], 0.587)
                gc = pool.tile([128, FREE], dt.bfloat16, name=f"gc_{nm}",
                               tag="gc", bufs=1)
                nc.scalar.mul(gc, ch[2], 0.114)
                g2 = pool.tile([128, FREE], dt.bfloat16, name=f"g2_{nm}",
                               tag="g2", bufs=1)
                nc.vector.tensor_add(g2, g1, gb)
                g3 = pool.tile([128, FREE], dt.bfloat16, name=f"g3_{nm}",
                               tag="g3", bufs=1)
                nc.vector.tensor_add(g3, g2, gc)

                g3v = g3.rearrange("p (r w) -> p r w", w=W)
                # gray rows are written straight into the band tile's center
                # slots (rows 3..6): bandA then only needs the halo DMAs
                bA = bands[nm + "A"]
                padv = bA.rearrange("p (r w) -> p r w", w=Wp)[:, PAD:PAD + RPP, :]
                # zero the 2 spare cols (0 and 519) so halo DMAs carry
                # defined bytes
                nc.vector.memset(
                    AP(bA.tensor, bA.offset + PAD * Wp,
                       [[BAND_LEN, 128], [Wp, RPP], [Wp - 1, 2]]),
                    0.0)
                # center cols: gray col w -> padded col w+COL0
                nc.vector.tensor_copy(out=padv[:, :, COL0:COL0 + W], in_=g3v)
                # reflect cols: padded col COL0-t = gray col t (t=1..3)
                nc.vector.tensor_copy(out=padv[:, :, 1:4], in_=g3v[:, :, 3:0:-1])
                # padded col COL0+W-1+t = gray col W-1-t
                nc.vector.tensor_copy(out=padv[:, :, 516:519],
                                      in_=g3v[:, :, 510:507:-1])

            # ---- halo construction, all SBUF->SBUF within the band ----
            # center slot s (gray row 4p+s) lives at band offset (3+s)*Wp
            for nm in ("t", "p"):
                qeng = nc.sync if nm == "p" else nc.scalar
                bA = bands[nm + "A"]
                pstride_b = bA.ap[0][0]
                # top halo: band[p][slots 0..2] <- band[p-1][center slots 1..3]
                qeng.dma_start(
                    out=AP(bA.tensor, bA.offset + 1 * pstride_b,
                           [[pstride_b, 127], [1, 3 * Wp]]),
                    in_=AP(bA.tensor, bA.offset + 4 * Wp,
                           [[pstride_b, 127], [1, 3 * Wp]]))
                # bottom halo: band[p][slots 7..9] <- band[p+1][center 0..2]
                qeng.dma_start(
                    out=AP(bA.tensor, bA.offset + 7 * Wp,
                           [[pstride_b, 127], [1, 3 * Wp]]),
                    in_=AP(bA.tensor, bA.offset + 1 * pstride_b + 3 * Wp,
                           [[pstride_b, 127], [1, 3 * Wp]]))
                # reflect edges: partition 0 top = gray rows 3,2,1 (center
                # slots 3,2,1); partition 127 bottom = gray rows 510,509,508
                # (center slots 2,1,0)
                for s_band, slot in ((0, 3), (1, 2), (2, 1)):
                    qeng.dma_start(
                        out=AP(bA.tensor, bA.offset + s_band * Wp,
                               [[pstride_b, 1], [1, Wp]]),
                        in_=AP(bA.tensor, bA.offset + (PAD + slot) * Wp,
                               [[pstride_b, 1], [1, Wp]]))
                for s_band, slot in ((7, 2), (8, 1), (9, 0)):
                    qeng.dma_start(
                        out=AP(bA.tensor,
                               bA.offset + 127 * pstride_b + s_band * Wp,
                               [[pstride_b, 1], [1, Wp]]),
                        in_=AP(bA.tensor,
                               bA.offset + 127 * pstride_b + (PAD + slot) * Wp,
                               [[pstride_b, 1], [1, Wp]]))
            # bandB = bandA shifted left one element (last element unused and
            # never read by any compute view)
            for nm in ("p", "t"):
                qeng = nc.sync if nm == "p" else nc.scalar
                bA, bB = bands[nm + "A"], bands[nm + "B"]
                qeng.dma_start(out=bB[:, 0:BAND_LEN - 1],
                               in_=bA[:, 1:BAND_LEN])

            # ---- main loop ----
            centers = {nm: band_view(bands[nm + "A"], PAD, COL0)
                       for nm in ("p", "t")}
            acc48 = pool.tile([128, max(n_off, 1)], dt.float32,
                              name="acc48", tag="acc48")
            nc.vector.memset(acc48, 0.0)
            ones = pool.tile([128, 1], dt.bfloat16, name="ones", tag="ones")
            nc.vector.memset(ones, 1.0)
            with tc.tile_pool(name="psum", bufs=1, space="PSUM") as ppool:
                prod = ppool.tile([128, 128], dt.float32, name="prod")
                sums = ppool.tile([1, 512], dt.float32, name="sums")
                offs = _offsets()[:n_off]
                # every 8th offset's cmpP sum goes to PE instead of ACT
                pe_sum_idx = {i for i in range(len(offs)) if i % 8 == 7}
                # a subset of offsets computes d = center - neighbor on the
                # (otherwise idle) GPSIMD engine, then binarizes on DVE with
                # tensor_scalar(is_gt, 0) in 4x mode — bf16 subtraction sign
                # is exact, so results are identical to a direct is_gt
                gp_n = int(_CACHE.get("gp_n", 8))
                gp_idx = {i for i in range(len(offs)) if i % 6 == 5}
                gp_idx = set(sorted(gp_idx)[:gp_n])
                for i, (di, dj) in enumerate(offs):
                    cmps = {}
                    for nm in ("p", "t"):
                        if dj % 2 == 0:
                            nb = band_view(bands[nm + "A"], PAD + di, COL0 + dj)
                        else:
                            nb = band_view(bands[nm + "B"], PAD + di,
                                           COL0 + dj - 1)
                        cmp = pool.tile([128, FREE], dt.bfloat16,
                                        name=f"cmp_{nm}_{i}", tag=f"cmp_{nm}",
                                        bufs=8)
                        if i in gp_idx:
                            dsub = pool.tile([128, FREE], dt.bfloat16,
                                             name=f"d_{nm}_{i}", tag=f"d_{nm}",
                                             bufs=2)
                            nc.gpsimd.tensor_tensor(
                                out=dsub.rearrange("p (r w) -> p r w", w=W),
                                in0=centers[nm], in1=nb, op=op.subtract)
                            nc.vector.tensor_scalar(
                                out=cmp, in0=dsub, scalar1=0.0, scalar2=None,
                                op0=op.is_gt)
                        else:
                            nc.vector.tensor_tensor(
                                out=cmp.rearrange("p (r w) -> p r w", w=W),
                                in0=centers[nm], in1=nb, op=op.is_gt)
                        cmps[nm] = cmp
                    if i in pe_sum_idx:
                        for c in range(FREE // 512):
                            nc.tensor.matmul(
                                sums[0:1, :], ones[:, 0:1],
                                cmps["p"][:, c * 512:(c + 1) * 512],
                                start=False, stop=False,
                                skip_group_check=True)
                    else:
                        dact = pool.tile([128, FREE], dt.bfloat16,
                                         name=f"dact_{i}", tag="dact", bufs=1)
                        nc.scalar.activation(
                            out=dact, in_=cmps["p"],
                            func=mybir.ActivationFunctionType.Copy,
                            accum_out=acc48[:, i:i + 1])
                    for c in range(FREE // 128):
                        nc.tensor.matmul(
                            prod[:, :],
                            cmps["p"][:, c * 128:(c + 1) * 128],
                            cmps["t"][:, c * 128:(c + 1) * 128],
                            start=(i == 0 and c == 0),
                            stop=(i == len(offs) - 1 and c == FREE // 128 - 1),
                            skip_group_check=True)
                    for c in range(FREE // 512):
                        nc.tensor.matmul(
                            sums[0:1, :], ones[:, 0:1],
                            cmps["t"][:, c * 512:(c + 1) * 512],
                            start=(i == 0 and c == 0),
                            stop=(i == len(offs) - 1 and c == FREE // 512 - 1),
                            skip_group_check=True)

                prod_sb = pool.tile([128, 128], dt.float32, name="prod_sb",
                                    tag="prod_sb")
                sums_sb = pool.tile([1, 512], dt.float32, name="sums_sb",
                                    tag="sums_sb")
                if n_off == 0:
                    nc.vector.memset(prod_sb, 0.0)
                    nc.vector.memset(sums_sb, 0.0)
                else:
                    nc.vector.tensor_copy(out=prod_sb, in_=prod)
                    nc.vector.tensor_copy(out=sums_sb, in_=sums)
                nc.sync.dma_start(out=acc48_out.ap(), in_=acc48)
                nc.sync.dma_start(out=prod_out.ap(), in_=prod_sb)
                nc.sync.dma_start(out=sums_out.ap(), in_=sums_sb)

    nc.finalize()
    return nc


def kernel(pred: np.ndarray, target: np.ndarray) -> np.ndarray:
    from concourse import bass_utils

    if "nc" not in _CACHE:
        _CACHE["nc"] = _build_bass()
    nc = _CACHE["nc"]

    pred = np.ascontiguousarray(pred, dtype=np.float32)
    target = np.ascontiguousarray(target, dtype=np.float32)
    in_maps = [
        {"pred": pred[b], "target": target[b]} for b in range(N_CORES)
    ]
    res = bass_utils.run_bass_kernel_spmd(nc, in_maps,
                                          core_ids=list(range(N_CORES)))
    total = 0.0
    for r in res.results:
        total += float(r["acc48_out"].astype(np.float64).sum())
        total += float(r["sums_out"].astype(np.float64).sum())
        total -= 2.0 * float(np.diag(r["prod_out"]).astype(np.float64).sum())
    mean = total / (B * N_OFF * H * W)
    return np.array(mean, dtype=np.float32)
